# revision 25
# baseline (speedup 1.0000x reference)
"""Multi-head attention Trainium2 kernel (8 NeuronCores, SPMD), v2.

Sharding: core c handles batch b = c//4 and heads [4*(c%4), 4*(c%4)+4).
Each core computes Q/K/V projections for its 4 heads, causal+biased
softmax attention, and a partial out-projection (its heads' columns of
wo). Host sums the 4 bf16 partials per batch and adds bo.

v2 design (vs v1 baseline):
  - attn_bias is applied MULTIPLICATIVELY: host ships ebs = exp(bias)
    in bf16 with causal/padding entries zeroed; device computes
    P = exp(S^T) * ebs.  This removes the fp32 PSUM tensor_add per
    score tile (the v1 DVE bottleneck) and makes masking exact zeros.
  - Scores for the 2 heads of a pair run as row-tiled matmul pairs
    (contraction rows 0-63 vs 64-127) so the K=64 matmuls share the PE.
  - exp() runs on [128, 1024] PSUM groups (2 banks) to amortize the
    ~352-cycle ACTIVATE overhead.
  - Normalization uses reciprocal_approx_fast (single DVE op) instead
    of the 8-cyc/elem iterative reciprocal.
  - Emission is software-pipelined: slot k issues scores(g_k) then
    PV(g_{k-1}); projection/out-proj matmul "filler" units are spread
    between attention slots so the PE never idles long enough to lose
    its HAM boost clock.
  - Partial out-proj results ship as bf16 (half the output DMA).
"""

import os
import sys
import numpy as np
from collections import deque

for _p in ("/opt/trn_rl_repo", "/root/.axon_site/_ro/trn_rl_repo"):
    if os.path.isdir(_p) and _p not in sys.path:
        sys.path.insert(0, _p)
        break


def _install_ntff_hook():
    """concourse's trace=True path wants antenv.axon_hooks, which the
    image's antenv lacks. Provide it (sys.modules shim) and register the
    ctypes NTFF hook from trn_agent_boot."""
    import types
    try:
        import antenv.axon_hooks  # noqa: F401
        return
    except ImportError:
        pass
    mod = types.ModuleType("antenv.axon_hooks")
    mod._hook = None
    mod.set_axon_ntff_profile_hook = lambda h: setattr(mod, "_hook", h)
    mod.get_axon_ntff_profile_hook = lambda: mod._hook
    try:
        import antenv
        sys.modules["antenv.axon_hooks"] = mod
        antenv.axon_hooks = mod
        from trn_agent_boot.trn_boot import _ntff_profile_via_ctypes
        so = "/opt/axon/libaxon_pjrt.so"
        if os.path.exists(so):
            mod._hook = _ntff_profile_via_ctypes(so)
    except Exception:
        pass


_install_ntff_hook()

# Problem constants (hardcoded per spec).
B, T, D, H = 2, 2048, 1024, 16
HD = D // H            # 64
NCORES = 8
NH = (B * H) // NCORES  # heads per core = 4
NPAIR = NH // 2        # head pairs per core = 2
DF = NH * HD           # 256  (per-core projection width)
VC = NH * (HD + 1)     # 260  (V with ones-column, 4 heads)
KTILE = 128            # d-dim tile for projections
NKT = D // KTILE       # 8
IC = 512               # query-position chunk (matmul moving dim)
NIC = T // IC          # 4
PJ = 128               # key-position tile (partition dim)
NJT = T // PJ          # 16
GJT = 2                # j-tiles per exp group
GW = GJT * IC          # free width of a score group = 1024
NBLK = 20              # sum_c GJT*(c+1) live groups per head

_STATE = {}
LAST_EXEC_NS = None
LAST_RESULTS = None


def _blk_idx(c, g):
    return c * (c + 1) + g


def _build_nc():
    import concourse.tile as tile
    from concourse import bacc, mybir
    from contextlib import ExitStack

    F32 = mybir.dt.float32
    BF16 = mybir.dt.bfloat16
    Exp = mybir.ActivationFunctionType.Exp
    Ident = mybir.ActivationFunctionType.Identity

    nc = bacc.Bacc("TRN2", target_bir_lowering=False, debug=False)

    xqT = nc.dram_tensor("xqT", [NIC, KTILE, NKT * IC], BF16,
                         kind="ExternalInput").ap()
    xkT = nc.dram_tensor("xkT", [NIC, KTILE, NKT * IC], BF16,
                         kind="ExternalInput").ap()
    xvT = nc.dram_tensor("xvT", [NIC, KTILE, NKT * IC], BF16,
                         kind="ExternalInput").ap()
    wqp = nc.dram_tensor("wqp", [KTILE, NKT * DF], BF16, kind="ExternalInput").ap()
    wkp = nc.dram_tensor("wkp", [KTILE, NKT * DF], BF16, kind="ExternalInput").ap()
    wvp = nc.dram_tensor("wvp", [KTILE, (NKT + 1) * VC], BF16,
                         kind="ExternalInput").ap()
    wot = nc.dram_tensor("wot", [DF, D], BF16, kind="ExternalInput").ap()
    bqk = nc.dram_tensor("bqk", [KTILE, 4], F32, kind="ExternalInput").ap()
    onesd = nc.dram_tensor("onesd", [128, IC], BF16, kind="ExternalInput").ap()
    ebsd = nc.dram_tensor("ebsd", [NH, NBLK, KTILE, GW], BF16,
                          kind="ExternalInput").ap()
    out = nc.dram_tensor("out", [T, D], BF16, kind="ExternalOutput").ap()

    with ExitStack() as ctx:
        tc = ctx.enter_context(tile.TileContext(nc))
        consts = ctx.enter_context(tc.tile_pool(name="consts", bufs=1))
        wpool = ctx.enter_context(tc.tile_pool(name="w", bufs=1))
        xpool = ctx.enter_context(tc.tile_pool(name="x", bufs=12))
        qkv = ctx.enter_context(tc.tile_pool(name="qkv", bufs=1))
        ebpool = ctx.enter_context(tc.tile_pool(name="ebs", bufs=8))
        ptpool = ctx.enter_context(tc.tile_pool(name="pt", bufs=6))
        rpool = ctx.enter_context(tc.tile_pool(name="r", bufs=8))
        outpool = ctx.enter_context(tc.tile_pool(name="outp", bufs=2))
        ppsum = ctx.enter_context(tc.tile_pool(name="ppsum", bufs=2, space="PSUM"))
        spsum = ctx.enter_context(tc.tile_pool(name="spsum", bufs=1, space="PSUM"))
        opsum = ctx.enter_context(tc.tile_pool(name="opsum", bufs=1, space="PSUM"))

        def dma_split(dst, src, nsplit):
            n = dst.shape[-1]
            step = n // nsplit
            for k in range(nsplit):
                ks = slice(k * step, (k + 1) * step if k < nsplit - 1 else n)
                nc.sync.dma_start(dst[:, ks], src[:, ks])

        # Chunk-0 activations + Q/K weights first (they gate the first
        # matmuls); everything else after.
        st0 = {}
        for tag, src in (("q", xqT), ("k", xkT), ("v", xvT)):
            sta = xpool.tile([128, 4 * IC], BF16, tag="x", name="xsta")
            stb = xpool.tile([128, 4 * IC], BF16, tag="x", name="xstb")
            st0[tag] = (sta, stb)
        wq_sb = wpool.tile([128, NKT * DF], BF16, tag="wq")
        wk_sb = wpool.tile([128, NKT * DF], BF16, tag="wk")
        wv_sb = wpool.tile([128, (NKT + 1) * VC], BF16, tag="wv")

        dma_split(st0["q"][0], xqT[0][:, 0:4 * IC], 4)
        dma_split(wq_sb, wqp, 4)
        dma_split(st0["k"][0], xkT[0][:, 0:4 * IC], 4)
        dma_split(wk_sb, wkp, 2)
        dma_split(st0["q"][1], xqT[0][:, 4 * IC:], 4)
        dma_split(st0["v"][0], xvT[0][:, 0:4 * IC], 4)
        dma_split(wv_sb, wvp, 2)
        dma_split(st0["k"][1], xkT[0][:, 4 * IC:], 4)
        dma_split(st0["v"][1], xvT[0][:, 4 * IC:], 4)

        # ones_x: row 0 = 1.0, rows 1..127 = 0.
        ones_x = consts.tile([128, IC], BF16, tag="ones")
        nc.sync.dma_start(ones_x, onesd)
        bqk_sb = wpool.tile([128, 4], F32, tag="bqk")
        nc.sync.dma_start(bqk_sb, bqk)
        wo_sb = [wpool.tile([128, D], BF16, tag=f"wo{m}", name=f"wo{m}")
                 for m in range(2)]
        for m in range(2):
            dma_split(wo_sb[m], wot[m * 128:(m + 1) * 128, :], 2)

        # Persistent activations.
        QT = [qkv.tile([128, T], BF16, tag=f"qt{m}", name=f"qt{m}") for m in range(2)]
        KT = [qkv.tile([128, T], BF16, tag=f"kt{m}", name=f"kt{m}") for m in range(2)]
        Vpp = [qkv.tile([128, VC], BF16, tag=f"vpp{j}", name=f"vpp{j}")
               for j in range(NJT)]
        OHT = [qkv.tile([128, T], BF16, tag=f"oht{m}", name=f"oht{m}")
               for m in range(2)]

        # ---------- projection / out-proj units (PE filler work) ----------
        def load_x():
            sta = xpool.tile([128, 4 * IC], BF16, tag="x", name="xsta")
            stb = xpool.tile([128, 4 * IC], BF16, tag="x", name="xstb")
            return (sta, stb)

        def start_x(st, src, c):
            dma_split(st[0], src[c][:, 0:4 * IC], 2)
            dma_split(st[1], src[c][:, 4 * IC:], 2)

        def unit_proj_qk(dst, w_sb, st, m, c, i_w):
            def emit():
                cs = slice(c * IC, (c + 1) * IC)
                ps = ppsum.tile([128, IC], F32, tag="pp")
                for k in range(NKT):
                    rhs = st[k // 4][:, (k % 4) * IC:(k % 4 + 1) * IC]
                    lhsT = w_sb[:, k * DF + m * 128: k * DF + (m + 1) * 128]
                    nc.tensor.matmul(ps, lhsT, rhs,
                                     start=(k == 0), stop=(k == NKT - 1))
                nc.scalar.activation(dst[m][:, cs], ps, Ident,
                                     bias=bqk_sb[:, 2 * i_w + m: 2 * i_w + m + 1])
            return emit

        def unit_proj_v(st, c, tt):
            def emit():
                jt = 4 * c + tt
                ps = ppsum.tile([128, VC], F32, tag="pp")
                for k in range(NKT + 1):
                    lhsT = (st[k // 4][:, (k % 4) * IC + tt * 128:
                                       (k % 4) * IC + (tt + 1) * 128]
                            if k < NKT else ones_x[:, 0:128])
                    rhs = wv_sb[:, k * VC:(k + 1) * VC]
                    nc.tensor.matmul(ps, lhsT, rhs,
                                     start=(k == 0), stop=(k == NKT))
                nc.vector.tensor_copy(Vpp[jt], ps)
            return emit

        def unit_outproj(tt):
            def emit():
                ts_ = slice(tt * 128, (tt + 1) * 128)
                ot = outpool.tile([128, D], BF16, tag="ot")
                for e in range(2):
                    es = slice(e * IC, (e + 1) * IC)
                    ps = ppsum.tile([128, IC], F32, tag="pp")
                    for m in range(2):
                        nc.tensor.matmul(ps,
                                         OHT[m][:, ts_],
                                         wo_sb[m][:, es],
                                         start=(m == 0), stop=(m == 1))
                    nc.vector.tensor_copy(ot[:, es], ps)
                for q4 in range(4):
                    qs = slice(q4 * 256, (q4 + 1) * 256)
                    nc.sync.dma_start(out[ts_, qs], ot[:, qs])
            return emit

        fillers = deque()

        def emit_fillers(n):
            for _ in range(n):
                if not fillers:
                    return
                fillers.popleft()()

        # ---------- attention machinery ----------
        ebt = {}     # (pair, s, g) -> prefetched ebs tile
        ptb = {}     # (pair, s) -> pt tile of the previous group
        ps2 = {}     # (pair, s) -> PV accumulator

        def prefetch_eb(pair, c, g):
            if g >= GJT * (c + 1):
                return
            for s in range(2):
                h = 2 * pair + s
                eb = ebpool.tile([128, GW], BF16, tag="eb", name="ebt")
                nc.sync.dma_start(eb, ebsd[h, _blk_idx(c, g)])
                ebt[(pair, s, g)] = eb

        def emit_pv(pair, c, g, last):
            for s in range(2):
                h = 2 * pair + s
                hcol = slice(h * (HD + 1), (h + 1) * (HD + 1))
                pt = ptb[(pair, s)]
                for u in range(GJT):
                    jt = GJT * g + u
                    nc.tensor.matmul(ps2[(pair, s)],
                                     Vpp[jt][:, hcol],
                                     pt[:, u * IC:(u + 1) * IC],
                                     start=(g == 0 and u == 0),
                                     stop=(last and u == GJT - 1))

        def emit_slot(pair, c, g):
            cs = slice(c * IC, (c + 1) * IC)
            sc = [spsum.tile([128, GW], F32, tag=f"sc{s}", name=f"sc{s}")
                  for s in range(2)]
            for u in range(GJT):
                jt = GJT * g + u
                js = slice(jt * PJ, (jt + 1) * PJ)
                for s in range(2):
                    rh = s * 64
                    nc.tensor.matmul(sc[s][:, u * IC:(u + 1) * IC],
                                     KT[pair][rh:rh + 64, js],
                                     QT[pair][rh:rh + 64, cs],
                                     start=True, stop=True)
            if g > 0:
                emit_pv(pair, c, g - 1, last=False)
            prefetch_eb(pair, c, g + 2)
            for s in range(2):
                pt = ptpool.tile([128, GW], BF16, tag="pt", name="ptt")
                nc.scalar.activation(pt, sc[s], Exp)
                nc.vector.tensor_mul(pt, pt, ebt.pop((pair, s, g)))
                ptb[(pair, s)] = pt

        def emit_norm(pair, c, split=1):
            recbs = []
            for s in range(2):
                den = rpool.tile([1, IC], F32, tag="den")
                nc.vector.tensor_copy(den, ps2[(pair, s)][HD:HD + 1, :])
                rec = rpool.tile([1, IC], F32, tag="rec")
                nc.vector.reciprocal_approx_fast(rec, den)
                recb = rpool.tile([1, IC], BF16, tag="recb")
                nc.vector.tensor_copy(recb, rec)
                recbs.append(recb)
            emit_fillers(1)
            reps = []
            for s in range(2):
                psr = ppsum.tile([64, IC], F32, tag="pp")
                nc.tensor.matmul(psr, ones_x[0:1, 0:64], recbs[s],
                                 start=True, stop=True)
                rep = rpool.tile([64, IC], BF16, tag="rep")
                nc.vector.tensor_copy(rep, psr)
                reps.append(rep)
            for v in range(split):
                vs = slice(v * (IC // split), (v + 1) * (IC // split))
                cvs = slice(c * IC + v * (IC // split),
                            c * IC + (v + 1) * (IC // split))
                for s in range(2):
                    rh = s * 64
                    nc.vector.tensor_mul(OHT[pair][rh:rh + 64, cvs],
                                         ps2[(pair, s)][0:HD, vs], reps[s][:, vs])

        # ---------- schedule ----------
        st_cur = st0

        # Projection units for chunk 0 run dense (nothing to overlap yet);
        # Q/K interleaved to match DMA arrival order.
        for m in range(2):
            unit_proj_qk(QT, wq_sb, st_cur["q"], m, 0, 0)()
            unit_proj_qk(KT, wk_sb, st_cur["k"], m, 0, 1)()
        for tt in range(4):
            unit_proj_v(st_cur["v"], 0, tt)()

        for c in range(NIC):
            # Queue filler units: projections for chunk c+1; out-proj for
            # finished t-chunks is deferred to the last attention chunk.
            if c + 1 < NIC:
                st_nxt = {}
                for tag, src in (("q", xqT), ("k", xkT), ("v", xvT)):
                    st_nxt[tag] = load_x()
                    start_x(st_nxt[tag], src, c + 1)
                for m in range(2):
                    fillers.append(unit_proj_qk(QT, wq_sb, st_nxt["q"], m, c + 1, 0))
                for m in range(2):
                    fillers.append(unit_proj_qk(KT, wk_sb, st_nxt["k"], m, c + 1, 1))
                for tt in range(4):
                    fillers.append(unit_proj_v(st_nxt["v"], c + 1, tt))
                st_cur = st_nxt
            if c == NIC - 1:
                for tt in range(4 * (NIC - 1)):
                    fillers.append(unit_outproj(tt))

            ng = GJT * (c + 1)
            chunk_fill = len(fillers)
            total_slots = NPAIR * ng
            slots_done = 0
            for pair in range(NPAIR):
                for s in range(2):
                    ps2[(pair, s)] = opsum.tile([HD + 1, IC], F32, tag=f"pv{s}",
                                                name=f"pv{s}")
                prefetch_eb(pair, c, 0)
                prefetch_eb(pair, c, 1)
                for g in range(ng):
                    emit_slot(pair, c, g)
                    slots_done += 1
                    # spread chunk fillers evenly over the chunk's slots
                    want = (chunk_fill * slots_done) // total_slots
                    done = chunk_fill - len(fillers)
                    if done < want:
                        emit_fillers(want - done)
                emit_pv(pair, c, ng - 1, last=True)
                emit_norm(pair, c, split=2 if c == NIC - 1 else 1)

        # Remaining fillers (late out-proj tiles) + final t-chunk.
        emit_fillers(len(fillers))
        for tt in range(4 * (NIC - 1), NJT):
            unit_outproj(tt)()

    nc.compile()
    return nc


def _bf16(x):
    import ml_dtypes
    return np.ascontiguousarray(np.asarray(x)).astype(ml_dtypes.bfloat16)


def _pack_w(wT, width):
    """[rows, width] -> zero-padded bf16 [128, ceil(rows/128)*width] laid out
    so SBUF partition p holds rows p, 128+p, ... back to back (contiguous
    per-partition DMA lines)."""
    nk = -(-wT.shape[0] // KTILE)
    outp = np.zeros((nk * KTILE, width), np.float32)
    outp[:wT.shape[0]] = wT
    return _bf16(outp.reshape(nk, KTILE, width).transpose(1, 0, 2)
                 .reshape(KTILE, nk * width))


def _prep_core(c, attn_bias, kp_mask, wq, bq, wk, bk, wv, bv, wo, xTs):
    b, hg = c // 4, c % 4
    rows = slice(DF * hg, DF * (hg + 1))
    qscale = np.float32(HD ** -0.5)

    wq_s = wq[rows].T * qscale           # [1024, 256]
    wk_s = wk[rows].T
    wv_aug = np.zeros((D + 1, VC), np.float32)
    wvT = wv[rows].T
    for kh in range(NH):
        wv_aug[:D, kh * (HD + 1):kh * (HD + 1) + HD] = \
            wvT[:, kh * HD:(kh + 1) * HD]
        wv_aug[D, kh * (HD + 1):kh * (HD + 1) + HD] = bv[rows][kh * HD:(kh + 1) * HD]
        wv_aug[D, kh * (HD + 1) + HD] = 1.0

    bqk = np.stack([bq[rows][:128] * qscale, bq[rows][128:] * qscale,
                    bk[rows][:128], bk[rows][128:]], axis=1)  # [128, 4]
    wot = _bf16(wo[:, rows].T)            # [256, 1024]

    # ebs = exp(bias^T) with causal / key-padding zeros, packed into the
    # per-(h, c, g) blocks the device loads: [NH, NBLK, 128, GJT*IC].
    import ml_dtypes
    ebs = np.empty((NH, NBLK, KTILE, GW), dtype=ml_dtypes.bfloat16)
    live = np.triu(np.ones((T, T), dtype=bool))  # [j, i]: live iff j <= i
    for h in range(NH):
        bt = attn_bias[b, NH * hg + h].T          # [j, i]
        E = np.exp(bt, dtype=np.float32)
        E[~live] = 0.0
        if kp_mask is not None and kp_mask[b].any():
            E[kp_mask[b], :] = 0.0
        Eb = E.astype(ml_dtypes.bfloat16)
        Er = Eb.reshape(NJT, PJ, NIC, IC)         # [jt, p, c, i]
        for cc in range(NIC):
            ngrp = GJT * (cc + 1)                 # exp groups for this chunk
            njt = GJT * ngrp                      # live j-tiles (= 4*(cc+1))
            blk = Er[:njt, :, cc, :].reshape(ngrp, GJT, PJ, IC)
            ebs[h, cc * (cc + 1):cc * (cc + 1) + ngrp] = \
                blk.transpose(0, 2, 1, 3).reshape(ngrp, PJ, GW)
    ones = np.zeros((128, IC), np.float32)
    ones[0, :] = 1.0
    return {
        "xqT": xTs[("q", b)], "xkT": xTs[("k", b)], "xvT": xTs[("v", b)],
        "wqp": _pack_w(wq_s, DF), "wkp": _pack_w(wk_s, DF),
        "wvp": _pack_w(wv_aug, VC),
        "wot": wot, "ebsd": ebs, "bqk": np.ascontiguousarray(bqk),
        "onesd": _bf16(ones),
    }


def kernel(query, key, value, attn_bias, key_padding_mask,
           wq, bq, wk, bk, wv, bv, wo, bo):
    global LAST_EXEC_NS, LAST_RESULTS
    from concourse.bass_utils import run_bass_kernel_spmd

    query = np.asarray(query, np.float32)
    key = np.asarray(key, np.float32)
    value = np.asarray(value, np.float32)
    attn_bias = np.asarray(attn_bias, np.float32)
    kp = np.asarray(key_padding_mask).astype(bool)
    wq, bq = np.asarray(wq, np.float32), np.asarray(bq, np.float32)
    wk, bk = np.asarray(wk, np.float32), np.asarray(bk, np.float32)
    wv, bv = np.asarray(wv, np.float32), np.asarray(bv, np.float32)
    wo, bo = np.asarray(wo, np.float32), np.asarray(bo, np.float32)

    if "nc" not in _STATE:
        _STATE["nc"] = _build_nc()
    nc = _STATE["nc"]

    xTs = {}
    for tag, arr in (("q", query), ("k", key), ("v", value)):
        for b in range(B):
            xT = _bf16(arr[b].T)                  # [D, T]
            xTs[(tag, b)] = np.ascontiguousarray(
                xT.reshape(NKT, KTILE, NIC, IC).transpose(2, 1, 0, 3)
                .reshape(NIC, KTILE, NKT * IC))

    from concurrent.futures import ThreadPoolExecutor
    with ThreadPoolExecutor(NCORES) as ex:
        in_maps = list(ex.map(
            lambda c: _prep_core(c, attn_bias, kp,
                                 wq, bq, wk, bk, wv, bv, wo, xTs),
            range(NCORES)))

    trace = os.environ.get("BASS_KERNEL_TRACE", "0") == "1"
    res = run_bass_kernel_spmd(nc, in_maps, core_ids=list(range(NCORES)),
                               trace=trace)
    LAST_EXEC_NS = res.exec_time_ns
    LAST_RESULTS = res

    outp = np.empty((B, T, D), np.float32)
    for b in range(B):
        acc = res.results[4 * b]["out"].astype(np.float32)
        for g in range(1, 4):
            acc = acc + res.results[4 * b + g]["out"].astype(np.float32)
        outp[b] = acc + bo
    return outp


# revision 29
# speedup vs baseline: 1.0065x; 1.0065x over previous
"""Multi-head attention Trainium2 kernel (8 NeuronCores, SPMD), v2.

Sharding: core c handles batch b = c//4 and heads [4*(c%4), 4*(c%4)+4).
Each core computes Q/K/V projections for its 4 heads, causal+biased
softmax attention, and a partial out-projection (its heads' columns of
wo). Host sums the 4 bf16 partials per batch and adds bo.

v2 design (vs v1 baseline):
  - attn_bias is applied MULTIPLICATIVELY: host ships ebs = exp(bias)
    in bf16 with causal/padding entries zeroed; device computes
    P = exp(S^T) * ebs.  This removes the fp32 PSUM tensor_add per
    score tile (the v1 DVE bottleneck) and makes masking exact zeros.
  - Scores for the 2 heads of a pair run as row-tiled matmul pairs
    (contraction rows 0-63 vs 64-127) so the K=64 matmuls share the PE.
  - exp() runs on [128, 1024] PSUM groups (2 banks) to amortize the
    ~352-cycle ACTIVATE overhead.
  - Normalization uses reciprocal_approx_fast (single DVE op) instead
    of the 8-cyc/elem iterative reciprocal.
  - Emission is software-pipelined: slot k issues scores(g_k) then
    PV(g_{k-1}); projection/out-proj matmul "filler" units are spread
    between attention slots so the PE never idles long enough to lose
    its HAM boost clock.
  - Partial out-proj results ship as bf16 (half the output DMA).
"""

import os
import sys
import numpy as np
from collections import deque

for _p in ("/opt/trn_rl_repo", "/root/.axon_site/_ro/trn_rl_repo"):
    if os.path.isdir(_p) and _p not in sys.path:
        sys.path.insert(0, _p)
        break


def _install_ntff_hook():
    """concourse's trace=True path wants antenv.axon_hooks, which the
    image's antenv lacks. Provide it (sys.modules shim) and register the
    ctypes NTFF hook from trn_agent_boot."""
    import types
    try:
        import antenv.axon_hooks  # noqa: F401
        return
    except ImportError:
        pass
    mod = types.ModuleType("antenv.axon_hooks")
    mod._hook = None
    mod.set_axon_ntff_profile_hook = lambda h: setattr(mod, "_hook", h)
    mod.get_axon_ntff_profile_hook = lambda: mod._hook
    try:
        import antenv
        sys.modules["antenv.axon_hooks"] = mod
        antenv.axon_hooks = mod
        from trn_agent_boot.trn_boot import _ntff_profile_via_ctypes
        so = "/opt/axon/libaxon_pjrt.so"
        if os.path.exists(so):
            mod._hook = _ntff_profile_via_ctypes(so)
    except Exception:
        pass


_install_ntff_hook()

# Problem constants (hardcoded per spec).
B, T, D, H = 2, 2048, 1024, 16
HD = D // H            # 64
NCORES = 8
NH = (B * H) // NCORES  # heads per core = 4
NPAIR = NH // 2        # head pairs per core = 2
DF = NH * HD           # 256  (per-core projection width)
VC = NH * (HD + 1)     # 260  (V with ones-column, 4 heads)
KTILE = 128            # d-dim tile for projections
NKT = D // KTILE       # 8
IC = 512               # query-position chunk (matmul moving dim)
NIC = T // IC          # 4
PJ = 128               # key-position tile (partition dim)
NJT = T // PJ          # 16
GJT = 2                # j-tiles per exp group
GW = GJT * IC          # free width of a score group = 1024
NBLK = 20              # sum_c GJT*(c+1) live groups per head

_STATE = {}
LAST_EXEC_NS = None
LAST_RESULTS = None


def _blk_idx(c, g):
    return c * (c + 1) + g


def _build_nc():
    import concourse.tile as tile
    from concourse import bacc, mybir
    from contextlib import ExitStack

    F32 = mybir.dt.float32
    BF16 = mybir.dt.bfloat16
    Exp = mybir.ActivationFunctionType.Exp
    Ident = mybir.ActivationFunctionType.Identity

    nc = bacc.Bacc("TRN2", target_bir_lowering=False, debug=False)

    xqT = nc.dram_tensor("xqT", [NIC, KTILE, NKT * IC], BF16,
                         kind="ExternalInput").ap()
    xkT = nc.dram_tensor("xkT", [NIC, KTILE, NKT * IC], BF16,
                         kind="ExternalInput").ap()
    xvT = nc.dram_tensor("xvT", [NIC, KTILE, NKT * IC], BF16,
                         kind="ExternalInput").ap()
    wqp = nc.dram_tensor("wqp", [KTILE, NKT * DF], BF16, kind="ExternalInput").ap()
    wkp = nc.dram_tensor("wkp", [KTILE, NKT * DF], BF16, kind="ExternalInput").ap()
    wvp = nc.dram_tensor("wvp", [KTILE, (NKT + 1) * VC], BF16,
                         kind="ExternalInput").ap()
    wot = nc.dram_tensor("wot", [DF, D], BF16, kind="ExternalInput").ap()
    bqk = nc.dram_tensor("bqk", [KTILE, 4], F32, kind="ExternalInput").ap()
    onesd = nc.dram_tensor("onesd", [128, IC], BF16, kind="ExternalInput").ap()
    ebsd = nc.dram_tensor("ebsd", [NH, NBLK, KTILE, GW], BF16,
                          kind="ExternalInput").ap()
    out = nc.dram_tensor("out", [T, D], BF16, kind="ExternalOutput").ap()

    with ExitStack() as ctx:
        tc = ctx.enter_context(tile.TileContext(nc))
        consts = ctx.enter_context(tc.tile_pool(name="consts", bufs=1))
        wpool = ctx.enter_context(tc.tile_pool(name="w", bufs=1))
        xpool = ctx.enter_context(tc.tile_pool(name="x", bufs=4))
        qkv = ctx.enter_context(tc.tile_pool(name="qkv", bufs=1))
        ebpool = ctx.enter_context(tc.tile_pool(name="ebs", bufs=8))
        ptpool = ctx.enter_context(tc.tile_pool(name="pt", bufs=6))
        rpool = ctx.enter_context(tc.tile_pool(name="r", bufs=8))
        outpool = ctx.enter_context(tc.tile_pool(name="outp", bufs=2))
        ppsum = ctx.enter_context(tc.tile_pool(name="ppsum", bufs=2, space="PSUM"))
        spsum = ctx.enter_context(tc.tile_pool(name="spsum", bufs=1, space="PSUM"))
        opsum = ctx.enter_context(tc.tile_pool(name="opsum", bufs=1, space="PSUM"))

        def dma_split(dst, src, nsplit):
            n = dst.shape[-1]
            step = n // nsplit
            for k in range(nsplit):
                ks = slice(k * step, (k + 1) * step if k < nsplit - 1 else n)
                nc.sync.dma_start(dst[:, ks], src[:, ks])

        # ones_x: row 0 = 1.0, rows 1..127 = 0.
        ones_x = consts.tile([128, IC], BF16, tag="ones")
        nc.sync.dma_start(ones_x, onesd)

        wq_sb = wpool.tile([128, NKT * DF], BF16, tag="wq")
        wk_sb = wpool.tile([128, NKT * DF], BF16, tag="wk")
        wv_sb = wpool.tile([128, (NKT + 1) * VC], BF16, tag="wv")
        dma_split(wq_sb, wqp, 2)
        dma_split(wk_sb, wkp, 2)
        dma_split(wv_sb, wvp, 2)
        bqk_sb = wpool.tile([128, 4], F32, tag="bqk")
        nc.sync.dma_start(bqk_sb, bqk)

        st0 = {}
        for tag, src in (("q", xqT), ("k", xkT), ("v", xvT)):
            st = xpool.tile([128, NKT * IC], BF16, tag="x", name="xst")
            dma_split(st, src[0], 4)
            st0[tag] = st

        wo_sb = [wpool.tile([128, D], BF16, tag=f"wo{m}", name=f"wo{m}")
                 for m in range(2)]
        for m in range(2):
            dma_split(wo_sb[m], wot[m * 128:(m + 1) * 128, :], 2)

        # Persistent activations.
        QT = [qkv.tile([128, T], BF16, tag=f"qt{m}", name=f"qt{m}") for m in range(2)]
        KT = [qkv.tile([128, T], BF16, tag=f"kt{m}", name=f"kt{m}") for m in range(2)]
        Vpp = [qkv.tile([128, VC], BF16, tag=f"vpp{j}", name=f"vpp{j}")
               for j in range(NJT)]
        OHT = [qkv.tile([128, T], BF16, tag=f"oht{m}", name=f"oht{m}")
               for m in range(2)]

        # ---------- projection / out-proj units (PE filler work) ----------
        def load_x():
            return xpool.tile([128, NKT * IC], BF16, tag="x", name="xst")

        def start_x(st, src, c):
            dma_split(st, src[c], 4)

        def unit_proj_qk(dst, w_sb, st, m, c, i_w):
            def emit():
                cs = slice(c * IC, (c + 1) * IC)
                ps = ppsum.tile([128, IC], F32, tag="pp")
                for k in range(NKT):
                    rhs = st[:, k * IC:(k + 1) * IC]
                    lhsT = w_sb[:, k * DF + m * 128: k * DF + (m + 1) * 128]
                    nc.tensor.matmul(ps, lhsT, rhs,
                                     start=(k == 0), stop=(k == NKT - 1))
                nc.scalar.activation(dst[m][:, cs], ps, Ident,
                                     bias=bqk_sb[:, 2 * i_w + m: 2 * i_w + m + 1])
            return emit

        def unit_proj_v(st, c, tt):
            def emit():
                jt = 4 * c + tt
                ps = ppsum.tile([128, VC], F32, tag="pp")
                for k in range(NKT + 1):
                    lhsT = (st[:, k * IC + tt * 128: k * IC + (tt + 1) * 128]
                            if k < NKT else ones_x[:, 0:128])
                    rhs = wv_sb[:, k * VC:(k + 1) * VC]
                    nc.tensor.matmul(ps, lhsT, rhs,
                                     start=(k == 0), stop=(k == NKT))
                nc.vector.tensor_copy(Vpp[jt], ps)
            return emit

        def unit_outproj(tt):
            def emit():
                ts_ = slice(tt * 128, (tt + 1) * 128)
                ot = outpool.tile([128, D], BF16, tag="ot")
                for e in range(2):
                    es = slice(e * IC, (e + 1) * IC)
                    ps = ppsum.tile([128, IC], F32, tag="pp")
                    for m in range(2):
                        nc.tensor.matmul(ps,
                                         OHT[m][:, ts_],
                                         wo_sb[m][:, es],
                                         start=(m == 0), stop=(m == 1))
                    nc.vector.tensor_copy(ot[:, es], ps)
                for q4 in range(4):
                    qs = slice(q4 * 256, (q4 + 1) * 256)
                    nc.sync.dma_start(out[ts_, qs], ot[:, qs])
            return emit

        fillers = deque()

        def emit_fillers(n):
            for _ in range(n):
                if not fillers:
                    return
                fillers.popleft()()

        # ---------- attention machinery ----------
        ebt = {}     # (pair, s, g) -> prefetched ebs tile
        ptb = {}     # (pair, s) -> pt tile of the previous group
        ps2 = {}     # (pair, s) -> PV accumulator

        def prefetch_eb(pair, c, g):
            if g >= GJT * (c + 1):
                return
            for s in range(2):
                h = 2 * pair + s
                eb = ebpool.tile([128, GW], BF16, tag="eb", name="ebt")
                nc.sync.dma_start(eb, ebsd[h, _blk_idx(c, g)])
                ebt[(pair, s, g)] = eb

        def emit_pv(pair, c, g, last):
            for s in range(2):
                h = 2 * pair + s
                hcol = slice(h * (HD + 1), (h + 1) * (HD + 1))
                pt = ptb[(pair, s)]
                for u in range(GJT):
                    jt = GJT * g + u
                    nc.tensor.matmul(ps2[(pair, s)],
                                     Vpp[jt][:, hcol],
                                     pt[:, u * IC:(u + 1) * IC],
                                     start=(g == 0 and u == 0),
                                     stop=(last and u == GJT - 1))

        def emit_slot(pair, c, g):
            cs = slice(c * IC, (c + 1) * IC)
            sc = [spsum.tile([128, GW], F32, tag=f"sc{s}", name=f"sc{s}")
                  for s in range(2)]
            for u in range(GJT):
                jt = GJT * g + u
                js = slice(jt * PJ, (jt + 1) * PJ)
                for s in range(2):
                    rh = s * 64
                    nc.tensor.matmul(sc[s][:, u * IC:(u + 1) * IC],
                                     KT[pair][rh:rh + 64, js],
                                     QT[pair][rh:rh + 64, cs],
                                     start=True, stop=True)
            if g > 0:
                emit_pv(pair, c, g - 1, last=False)
            prefetch_eb(pair, c, g + 2)
            for s in range(2):
                pt = ptpool.tile([128, GW], BF16, tag="pt", name="ptt")
                nc.scalar.activation(pt, sc[s], Exp)
                nc.vector.tensor_mul(pt, pt, ebt.pop((pair, s, g)))
                ptb[(pair, s)] = pt

        def emit_norm(pair, c, split=1):
            recbs = []
            for s in range(2):
                den = rpool.tile([1, IC], F32, tag="den")
                nc.vector.tensor_copy(den, ps2[(pair, s)][HD:HD + 1, :])
                rec = rpool.tile([1, IC], F32, tag="rec")
                nc.vector.reciprocal_approx_fast(rec, den)
                recb = rpool.tile([1, IC], BF16, tag="recb")
                nc.vector.tensor_copy(recb, rec)
                recbs.append(recb)
            emit_fillers(1)
            reps = []
            for s in range(2):
                psr = ppsum.tile([64, IC], F32, tag="pp")
                nc.tensor.matmul(psr, ones_x[0:1, 0:64], recbs[s],
                                 start=True, stop=True)
                rep = rpool.tile([64, IC], BF16, tag="rep")
                nc.vector.tensor_copy(rep, psr)
                reps.append(rep)
            for v in range(split):
                vs = slice(v * (IC // split), (v + 1) * (IC // split))
                cvs = slice(c * IC + v * (IC // split),
                            c * IC + (v + 1) * (IC // split))
                for s in range(2):
                    rh = s * 64
                    nc.vector.tensor_mul(OHT[pair][rh:rh + 64, cvs],
                                         ps2[(pair, s)][0:HD, vs], reps[s][:, vs])

        # ---------- schedule ----------
        st_cur = st0

        # Projection units for chunk 0 run dense (nothing to overlap yet).
        for m in range(2):
            unit_proj_qk(QT, wq_sb, st_cur["q"], m, 0, 0)()
        for m in range(2):
            unit_proj_qk(KT, wk_sb, st_cur["k"], m, 0, 1)()
        for tt in range(4):
            unit_proj_v(st_cur["v"], 0, tt)()

        for c in range(NIC):
            # Queue filler units: projections for chunk c+1; out-proj for
            # finished t-chunks is deferred to the last attention chunk.
            if c + 1 < NIC:
                st_nxt = {}
                for tag, src in (("q", xqT), ("k", xkT), ("v", xvT)):
                    st_nxt[tag] = load_x()
                    start_x(st_nxt[tag], src, c + 1)
                for m in range(2):
                    fillers.append(unit_proj_qk(QT, wq_sb, st_nxt["q"], m, c + 1, 0))
                for m in range(2):
                    fillers.append(unit_proj_qk(KT, wk_sb, st_nxt["k"], m, c + 1, 1))
                for tt in range(4):
                    fillers.append(unit_proj_v(st_nxt["v"], c + 1, tt))
                st_cur = st_nxt
            if c == NIC - 1:
                for tt in range(4 * (NIC - 1)):
                    fillers.append(unit_outproj(tt))

            ng = GJT * (c + 1)
            chunk_fill = len(fillers)
            total_slots = NPAIR * ng
            slots_done = 0
            for pair in range(NPAIR):
                for s in range(2):
                    ps2[(pair, s)] = opsum.tile([HD + 1, IC], F32, tag=f"pv{s}",
                                                name=f"pv{s}")
                prefetch_eb(pair, c, 0)
                prefetch_eb(pair, c, 1)
                for g in range(ng):
                    emit_slot(pair, c, g)
                    slots_done += 1
                    # spread chunk fillers evenly over the chunk's slots
                    want = (chunk_fill * slots_done) // total_slots
                    done = chunk_fill - len(fillers)
                    if done < want:
                        emit_fillers(want - done)
                emit_pv(pair, c, ng - 1, last=True)
                emit_norm(pair, c, split=2 if c == NIC - 1 else 1)

        # Remaining fillers (late out-proj tiles) + final t-chunk.
        emit_fillers(len(fillers))
        for tt in range(4 * (NIC - 1), NJT):
            unit_outproj(tt)()

    nc.compile()
    return nc


def _bf16(x):
    import ml_dtypes
    return np.ascontiguousarray(np.asarray(x)).astype(ml_dtypes.bfloat16)


def _pack_w(wT, width):
    """[rows, width] -> zero-padded bf16 [128, ceil(rows/128)*width] laid out
    so SBUF partition p holds rows p, 128+p, ... back to back (contiguous
    per-partition DMA lines)."""
    nk = -(-wT.shape[0] // KTILE)
    outp = np.zeros((nk * KTILE, width), np.float32)
    outp[:wT.shape[0]] = wT
    return _bf16(outp.reshape(nk, KTILE, width).transpose(1, 0, 2)
                 .reshape(KTILE, nk * width))


def _prep_core(c, attn_bias, kp_mask, wq, bq, wk, bk, wv, bv, wo, xTs):
    b, hg = c // 4, c % 4
    rows = slice(DF * hg, DF * (hg + 1))
    qscale = np.float32(HD ** -0.5)

    wq_s = wq[rows].T * qscale           # [1024, 256]
    wk_s = wk[rows].T
    wv_aug = np.zeros((D + 1, VC), np.float32)
    wvT = wv[rows].T
    for kh in range(NH):
        wv_aug[:D, kh * (HD + 1):kh * (HD + 1) + HD] = \
            wvT[:, kh * HD:(kh + 1) * HD]
        wv_aug[D, kh * (HD + 1):kh * (HD + 1) + HD] = bv[rows][kh * HD:(kh + 1) * HD]
        wv_aug[D, kh * (HD + 1) + HD] = 1.0

    bqk = np.stack([bq[rows][:128] * qscale, bq[rows][128:] * qscale,
                    bk[rows][:128], bk[rows][128:]], axis=1)  # [128, 4]
    wot = _bf16(wo[:, rows].T)            # [256, 1024]

    # ebs = exp(bias^T) with causal / key-padding zeros, packed into the
    # per-(h, c, g) blocks the device loads: [NH, NBLK, 128, GJT*IC].
    import ml_dtypes
    ebs = np.empty((NH, NBLK, KTILE, GW), dtype=ml_dtypes.bfloat16)
    live = np.triu(np.ones((T, T), dtype=bool))  # [j, i]: live iff j <= i
    for h in range(NH):
        bt = attn_bias[b, NH * hg + h].T          # [j, i]
        E = np.exp(bt, dtype=np.float32)
        E[~live] = 0.0
        if kp_mask is not None and kp_mask[b].any():
            E[kp_mask[b], :] = 0.0
        Eb = E.astype(ml_dtypes.bfloat16)
        Er = Eb.reshape(NJT, PJ, NIC, IC)         # [jt, p, c, i]
        for cc in range(NIC):
            ngrp = GJT * (cc + 1)                 # exp groups for this chunk
            njt = GJT * ngrp                      # live j-tiles (= 4*(cc+1))
            blk = Er[:njt, :, cc, :].reshape(ngrp, GJT, PJ, IC)
            ebs[h, cc * (cc + 1):cc * (cc + 1) + ngrp] = \
                blk.transpose(0, 2, 1, 3).reshape(ngrp, PJ, GW)
    ones = np.zeros((128, IC), np.float32)
    ones[0, :] = 1.0
    return {
        "xqT": xTs[("q", b)], "xkT": xTs[("k", b)], "xvT": xTs[("v", b)],
        "wqp": _pack_w(wq_s, DF), "wkp": _pack_w(wk_s, DF),
        "wvp": _pack_w(wv_aug, VC),
        "wot": wot, "ebsd": ebs, "bqk": np.ascontiguousarray(bqk),
        "onesd": _bf16(ones),
    }


def kernel(query, key, value, attn_bias, key_padding_mask,
           wq, bq, wk, bk, wv, bv, wo, bo):
    global LAST_EXEC_NS, LAST_RESULTS
    from concourse.bass_utils import run_bass_kernel_spmd

    query = np.asarray(query, np.float32)
    key = np.asarray(key, np.float32)
    value = np.asarray(value, np.float32)
    attn_bias = np.asarray(attn_bias, np.float32)
    kp = np.asarray(key_padding_mask).astype(bool)
    wq, bq = np.asarray(wq, np.float32), np.asarray(bq, np.float32)
    wk, bk = np.asarray(wk, np.float32), np.asarray(bk, np.float32)
    wv, bv = np.asarray(wv, np.float32), np.asarray(bv, np.float32)
    wo, bo = np.asarray(wo, np.float32), np.asarray(bo, np.float32)

    if "nc" not in _STATE:
        _STATE["nc"] = _build_nc()
    nc = _STATE["nc"]

    xTs = {}
    for tag, arr in (("q", query), ("k", key), ("v", value)):
        for b in range(B):
            xT = _bf16(arr[b].T)                  # [D, T]
            xTs[(tag, b)] = np.ascontiguousarray(
                xT.reshape(NKT, KTILE, NIC, IC).transpose(2, 1, 0, 3)
                .reshape(NIC, KTILE, NKT * IC))

    from concurrent.futures import ThreadPoolExecutor
    with ThreadPoolExecutor(NCORES) as ex:
        in_maps = list(ex.map(
            lambda c: _prep_core(c, attn_bias, kp,
                                 wq, bq, wk, bk, wv, bv, wo, xTs),
            range(NCORES)))

    trace = os.environ.get("BASS_KERNEL_TRACE", "0") == "1"
    res = run_bass_kernel_spmd(nc, in_maps, core_ids=list(range(NCORES)),
                               trace=trace)
    LAST_EXEC_NS = res.exec_time_ns
    LAST_RESULTS = res

    outp = np.empty((B, T, D), np.float32)
    for b in range(B):
        acc = res.results[4 * b]["out"].astype(np.float32)
        for g in range(1, 4):
            acc = acc + res.results[4 * b + g]["out"].astype(np.float32)
        outp[b] = acc + bo
    return outp


# revision 30
# speedup vs baseline: 1.1328x; 1.1256x over previous
"""Multi-head attention Trainium2 kernel (8 NeuronCores, SPMD), v2.

Sharding: core c handles batch b = c//4 and heads [4*(c%4), 4*(c%4)+4).
Each core computes Q/K/V projections for its 4 heads, causal+biased
softmax attention, and a partial out-projection (its heads' columns of
wo). Host sums the 4 bf16 partials per batch and adds bo.

v2 design (vs v1 baseline):
  - attn_bias is applied MULTIPLICATIVELY: host ships ebs = exp(bias)
    in bf16 with causal/padding entries zeroed; device computes
    P = exp(S^T) * ebs.  This removes the fp32 PSUM tensor_add per
    score tile (the v1 DVE bottleneck) and makes masking exact zeros.
  - Scores for the 2 heads of a pair run as row-tiled matmul pairs
    (contraction rows 0-63 vs 64-127) so the K=64 matmuls share the PE.
  - exp() runs on [128, 1024] PSUM groups (2 banks) to amortize the
    ~352-cycle ACTIVATE overhead.
  - Normalization uses reciprocal_approx_fast (single DVE op) instead
    of the 8-cyc/elem iterative reciprocal.
  - Emission is software-pipelined: slot k issues scores(g_k) then
    PV(g_{k-1}); projection/out-proj matmul "filler" units are spread
    between attention slots so the PE never idles long enough to lose
    its HAM boost clock.
  - Partial out-proj results ship as bf16 (half the output DMA).
"""

import os
import sys
import numpy as np
from collections import deque

for _p in ("/opt/trn_rl_repo", "/root/.axon_site/_ro/trn_rl_repo"):
    if os.path.isdir(_p) and _p not in sys.path:
        sys.path.insert(0, _p)
        break


def _install_ntff_hook():
    """concourse's trace=True path wants antenv.axon_hooks, which the
    image's antenv lacks. Provide it (sys.modules shim) and register the
    ctypes NTFF hook from trn_agent_boot."""
    import types
    try:
        import antenv.axon_hooks  # noqa: F401
        return
    except ImportError:
        pass
    mod = types.ModuleType("antenv.axon_hooks")
    mod._hook = None
    mod.set_axon_ntff_profile_hook = lambda h: setattr(mod, "_hook", h)
    mod.get_axon_ntff_profile_hook = lambda: mod._hook
    try:
        import antenv
        sys.modules["antenv.axon_hooks"] = mod
        antenv.axon_hooks = mod
        from trn_agent_boot.trn_boot import _ntff_profile_via_ctypes
        so = "/opt/axon/libaxon_pjrt.so"
        if os.path.exists(so):
            mod._hook = _ntff_profile_via_ctypes(so)
    except Exception:
        pass


_install_ntff_hook()

# Problem constants (hardcoded per spec).
B, T, D, H = 2, 2048, 1024, 16
HD = D // H            # 64
NCORES = 8
NH = (B * H) // NCORES  # heads per core = 4
NPAIR = NH // 2        # head pairs per core = 2
DF = NH * HD           # 256  (per-core projection width)
VC = NH * (HD + 1)     # 260  (V with ones-column, 4 heads)
KTILE = 128            # d-dim tile for projections
NKT = D // KTILE       # 8
IC = 512               # query-position chunk (matmul moving dim)
NIC = T // IC          # 4
PJ = 128               # key-position tile (partition dim)
NJT = T // PJ          # 16
GJT = 2                # j-tiles per exp group
GW = GJT * IC          # free width of a score group = 1024
NBLK = 20              # sum_c GJT*(c+1) live groups per head

_STATE = {}
LAST_EXEC_NS = None
LAST_RESULTS = None


def _blk_idx(c, g):
    return c * (c + 1) + g


def _build_nc():
    import concourse.tile as tile
    from concourse import bacc, mybir
    from contextlib import ExitStack

    F32 = mybir.dt.float32
    BF16 = mybir.dt.bfloat16
    Exp = mybir.ActivationFunctionType.Exp
    Ident = mybir.ActivationFunctionType.Identity

    nc = bacc.Bacc("TRN2", target_bir_lowering=False, debug=False)

    xqT = nc.dram_tensor("xqT", [NIC, KTILE, NKT * IC], BF16,
                         kind="ExternalInput").ap()
    xkT = nc.dram_tensor("xkT", [NIC, KTILE, NKT * IC], BF16,
                         kind="ExternalInput").ap()
    xvT = nc.dram_tensor("xvT", [NIC, KTILE, NKT * IC], BF16,
                         kind="ExternalInput").ap()
    wqp = nc.dram_tensor("wqp", [KTILE, NKT * DF], BF16, kind="ExternalInput").ap()
    wkp = nc.dram_tensor("wkp", [KTILE, NKT * DF], BF16, kind="ExternalInput").ap()
    wvp = nc.dram_tensor("wvp", [KTILE, (NKT + 1) * VC], BF16,
                         kind="ExternalInput").ap()
    wot = nc.dram_tensor("wot", [DF, D], BF16, kind="ExternalInput").ap()
    bqk = nc.dram_tensor("bqk", [KTILE, 4], F32, kind="ExternalInput").ap()
    onesd = nc.dram_tensor("onesd", [128, IC], BF16, kind="ExternalInput").ap()
    ebsd = nc.dram_tensor("ebsd", [NH, NBLK, KTILE, GW], BF16,
                          kind="ExternalInput").ap()
    out = nc.dram_tensor("out", [T, D], BF16, kind="ExternalOutput").ap()

    with ExitStack() as ctx:
        tc = ctx.enter_context(tile.TileContext(nc))
        consts = ctx.enter_context(tc.tile_pool(name="consts", bufs=1))
        wpool = ctx.enter_context(tc.tile_pool(name="w", bufs=1))
        xpool = ctx.enter_context(tc.tile_pool(name="x", bufs=4))
        qkv = ctx.enter_context(tc.tile_pool(name="qkv", bufs=1))
        ebpool = ctx.enter_context(tc.tile_pool(name="ebs", bufs=8))
        ptpool = ctx.enter_context(tc.tile_pool(name="pt", bufs=6))
        rpool = ctx.enter_context(tc.tile_pool(name="r", bufs=8))
        outpool = ctx.enter_context(tc.tile_pool(name="outp", bufs=2))
        ppsum = ctx.enter_context(tc.tile_pool(name="ppsum", bufs=2, space="PSUM"))
        spsum = ctx.enter_context(tc.tile_pool(name="spsum", bufs=1, space="PSUM"))
        opsum = ctx.enter_context(tc.tile_pool(name="opsum", bufs=1, space="PSUM"))

        def dma_split(dst, src, nsplit):
            n = dst.shape[-1]
            step = n // nsplit
            for k in range(nsplit):
                ks = slice(k * step, (k + 1) * step if k < nsplit - 1 else n)
                nc.sync.dma_start(dst[:, ks], src[:, ks])

        # ones_x: row 0 = 1.0, rows 1..127 = 0.
        ones_x = consts.tile([128, IC], BF16, tag="ones")
        nc.sync.dma_start(ones_x, onesd)

        wq_sb = wpool.tile([128, NKT * DF], BF16, tag="wq")
        wk_sb = wpool.tile([128, NKT * DF], BF16, tag="wk")
        wv_sb = wpool.tile([128, (NKT + 1) * VC], BF16, tag="wv")
        nc.sync.dma_start(wq_sb, wqp)
        nc.sync.dma_start(wk_sb, wkp)
        nc.sync.dma_start(wv_sb, wvp)
        bqk_sb = wpool.tile([128, 4], F32, tag="bqk")
        nc.sync.dma_start(bqk_sb, bqk)

        st0 = {}
        for tag, src in (("q", xqT), ("k", xkT), ("v", xvT)):
            st = xpool.tile([128, NKT * IC], BF16, tag="x", name="xst")
            dma_split(st, src[0], 2)
            st0[tag] = st

        wo_sb = [wpool.tile([128, D], BF16, tag=f"wo{m}", name=f"wo{m}")
                 for m in range(2)]
        for m in range(2):
            nc.sync.dma_start(wo_sb[m], wot[m * 128:(m + 1) * 128, :])

        # Persistent activations.
        QT = [qkv.tile([128, T], BF16, tag=f"qt{m}", name=f"qt{m}") for m in range(2)]
        KT = [qkv.tile([128, T], BF16, tag=f"kt{m}", name=f"kt{m}") for m in range(2)]
        Vpp = [qkv.tile([128, VC], BF16, tag=f"vpp{j}", name=f"vpp{j}")
               for j in range(NJT)]
        OHT = [qkv.tile([128, T], BF16, tag=f"oht{m}", name=f"oht{m}")
               for m in range(2)]

        # ---------- projection / out-proj units (PE filler work) ----------
        def load_x():
            return xpool.tile([128, NKT * IC], BF16, tag="x", name="xst")

        def start_x(st, src, c):
            nc.sync.dma_start(st, src[c])

        def unit_proj_qk(dst, w_sb, st, m, c, i_w):
            def emit():
                cs = slice(c * IC, (c + 1) * IC)
                ps = ppsum.tile([128, IC], F32, tag="pp")
                for k in range(NKT):
                    rhs = st[:, k * IC:(k + 1) * IC]
                    lhsT = w_sb[:, k * DF + m * 128: k * DF + (m + 1) * 128]
                    nc.tensor.matmul(ps, lhsT, rhs,
                                     start=(k == 0), stop=(k == NKT - 1))
                nc.scalar.activation(dst[m][:, cs], ps, Ident,
                                     bias=bqk_sb[:, 2 * i_w + m: 2 * i_w + m + 1])
            return emit

        def unit_proj_v(st, c, tt):
            def emit():
                jt = 4 * c + tt
                ps = ppsum.tile([128, VC], F32, tag="pp")
                for k in range(NKT + 1):
                    lhsT = (st[:, k * IC + tt * 128: k * IC + (tt + 1) * 128]
                            if k < NKT else ones_x[:, 0:128])
                    rhs = wv_sb[:, k * VC:(k + 1) * VC]
                    nc.tensor.matmul(ps, lhsT, rhs,
                                     start=(k == 0), stop=(k == NKT))
                nc.vector.tensor_copy(Vpp[jt], ps)
            return emit

        def unit_outproj(tt):
            def emit():
                ts_ = slice(tt * 128, (tt + 1) * 128)
                ot = outpool.tile([128, D], BF16, tag="ot")
                for e in range(2):
                    es = slice(e * IC, (e + 1) * IC)
                    ps = ppsum.tile([128, IC], F32, tag="pp")
                    for m in range(2):
                        nc.tensor.matmul(ps,
                                         OHT[m][:, ts_],
                                         wo_sb[m][:, es],
                                         start=(m == 0), stop=(m == 1))
                    nc.vector.tensor_copy(ot[:, es], ps)
                nc.sync.dma_start(out[ts_, :], ot)
            return emit

        fillers = deque()

        def emit_fillers(n):
            for _ in range(n):
                if not fillers:
                    return
                fillers.popleft()()

        # ---------- attention machinery ----------
        ebt = {}     # (pair, s, g) -> prefetched ebs tile
        ptb = {}     # (pair, s) -> pt tile of the previous group
        ps2 = {}     # (pair, s) -> PV accumulator

        def prefetch_eb(pair, c, g):
            if g >= GJT * (c + 1):
                return
            for s in range(2):
                h = 2 * pair + s
                eb = ebpool.tile([128, GW], BF16, tag="eb", name="ebt")
                nc.sync.dma_start(eb, ebsd[h, _blk_idx(c, g)])
                ebt[(pair, s, g)] = eb

        def emit_pv(pair, c, g, last):
            for s in range(2):
                h = 2 * pair + s
                hcol = slice(h * (HD + 1), (h + 1) * (HD + 1))
                pt = ptb[(pair, s)]
                for u in range(GJT):
                    jt = GJT * g + u
                    nc.tensor.matmul(ps2[(pair, s)],
                                     Vpp[jt][:, hcol],
                                     pt[:, u * IC:(u + 1) * IC],
                                     start=(g == 0 and u == 0),
                                     stop=(last and u == GJT - 1))

        def emit_slot(pair, c, g):
            cs = slice(c * IC, (c + 1) * IC)
            sc = [spsum.tile([128, GW], F32, tag=f"sc{s}", name=f"sc{s}")
                  for s in range(2)]
            for u in range(GJT):
                jt = GJT * g + u
                js = slice(jt * PJ, (jt + 1) * PJ)
                for s in range(2):
                    rh = s * 64
                    nc.tensor.matmul(sc[s][:, u * IC:(u + 1) * IC],
                                     KT[pair][rh:rh + 64, js],
                                     QT[pair][rh:rh + 64, cs],
                                     start=True, stop=True)
            if g > 0:
                emit_pv(pair, c, g - 1, last=False)
            prefetch_eb(pair, c, g + 2)
            for s in range(2):
                pt = ptpool.tile([128, GW], BF16, tag="pt", name="ptt")
                nc.scalar.activation(pt, sc[s], Exp)
                nc.vector.tensor_mul(pt, pt, ebt.pop((pair, s, g)))
                ptb[(pair, s)] = pt

        def emit_norm(pair, c, split=1):
            recbs = []
            for s in range(2):
                den = rpool.tile([1, IC], F32, tag="den")
                nc.vector.tensor_copy(den, ps2[(pair, s)][HD:HD + 1, :])
                rec = rpool.tile([1, IC], F32, tag="rec")
                nc.vector.reciprocal_approx_fast(rec, den)
                recb = rpool.tile([1, IC], BF16, tag="recb")
                nc.vector.tensor_copy(recb, rec)
                recbs.append(recb)
            emit_fillers(1)
            reps = []
            for s in range(2):
                psr = ppsum.tile([64, IC], F32, tag="pp")
                nc.tensor.matmul(psr, ones_x[0:1, 0:64], recbs[s],
                                 start=True, stop=True)
                rep = rpool.tile([64, IC], BF16, tag="rep")
                nc.vector.tensor_copy(rep, psr)
                reps.append(rep)
            for v in range(split):
                vs = slice(v * (IC // split), (v + 1) * (IC // split))
                cvs = slice(c * IC + v * (IC // split),
                            c * IC + (v + 1) * (IC // split))
                for s in range(2):
                    rh = s * 64
                    nc.vector.tensor_mul(OHT[pair][rh:rh + 64, cvs],
                                         ps2[(pair, s)][0:HD, vs], reps[s][:, vs])

        # ---------- schedule ----------
        st_cur = st0

        # Projection units for chunk 0 run dense (nothing to overlap yet).
        for m in range(2):
            unit_proj_qk(QT, wq_sb, st_cur["q"], m, 0, 0)()
        for m in range(2):
            unit_proj_qk(KT, wk_sb, st_cur["k"], m, 0, 1)()
        for tt in range(4):
            unit_proj_v(st_cur["v"], 0, tt)()

        for c in range(NIC):
            # Queue filler units: projections for chunk c+1; out-proj for
            # finished t-chunks is deferred to the last attention chunk.
            if c + 1 < NIC:
                st_nxt = {}
                for tag, src in (("q", xqT), ("k", xkT), ("v", xvT)):
                    st_nxt[tag] = load_x()
                    start_x(st_nxt[tag], src, c + 1)
                for m in range(2):
                    fillers.append(unit_proj_qk(QT, wq_sb, st_nxt["q"], m, c + 1, 0))
                for m in range(2):
                    fillers.append(unit_proj_qk(KT, wk_sb, st_nxt["k"], m, c + 1, 1))
                for tt in range(4):
                    fillers.append(unit_proj_v(st_nxt["v"], c + 1, tt))
                st_cur = st_nxt
            if c == NIC - 1:
                for tt in range(4 * (NIC - 1)):
                    fillers.append(unit_outproj(tt))

            ng = GJT * (c + 1)
            chunk_fill = len(fillers)
            total_slots = NPAIR * ng
            slots_done = 0
            for pair in range(NPAIR):
                for s in range(2):
                    ps2[(pair, s)] = opsum.tile([HD + 1, IC], F32, tag=f"pv{s}",
                                                name=f"pv{s}")
                prefetch_eb(pair, c, 0)
                prefetch_eb(pair, c, 1)
                for g in range(ng):
                    emit_slot(pair, c, g)
                    slots_done += 1
                    # spread chunk fillers evenly over the chunk's slots
                    want = (chunk_fill * slots_done) // total_slots
                    done = chunk_fill - len(fillers)
                    if done < want:
                        emit_fillers(want - done)
                emit_pv(pair, c, ng - 1, last=True)
                emit_norm(pair, c, split=2 if c == NIC - 1 else 1)

        # Remaining fillers (late out-proj tiles) + final t-chunk.
        emit_fillers(len(fillers))
        for tt in range(4 * (NIC - 1), NJT):
            unit_outproj(tt)()

    nc.compile()
    return nc


def _bf16(x):
    import ml_dtypes
    return np.ascontiguousarray(np.asarray(x)).astype(ml_dtypes.bfloat16)


def _pack_w(wT, width):
    """[rows, width] -> zero-padded bf16 [128, ceil(rows/128)*width] laid out
    so SBUF partition p holds rows p, 128+p, ... back to back (contiguous
    per-partition DMA lines)."""
    nk = -(-wT.shape[0] // KTILE)
    outp = np.zeros((nk * KTILE, width), np.float32)
    outp[:wT.shape[0]] = wT
    return _bf16(outp.reshape(nk, KTILE, width).transpose(1, 0, 2)
                 .reshape(KTILE, nk * width))


def _prep_core(c, attn_bias, kp_mask, wq, bq, wk, bk, wv, bv, wo, xTs):
    b, hg = c // 4, c % 4
    rows = slice(DF * hg, DF * (hg + 1))
    qscale = np.float32(HD ** -0.5)

    wq_s = wq[rows].T * qscale           # [1024, 256]
    wk_s = wk[rows].T
    wv_aug = np.zeros((D + 1, VC), np.float32)
    wvT = wv[rows].T
    for kh in range(NH):
        wv_aug[:D, kh * (HD + 1):kh * (HD + 1) + HD] = \
            wvT[:, kh * HD:(kh + 1) * HD]
        wv_aug[D, kh * (HD + 1):kh * (HD + 1) + HD] = bv[rows][kh * HD:(kh + 1) * HD]
        wv_aug[D, kh * (HD + 1) + HD] = 1.0

    bqk = np.stack([bq[rows][:128] * qscale, bq[rows][128:] * qscale,
                    bk[rows][:128], bk[rows][128:]], axis=1)  # [128, 4]
    wot = _bf16(wo[:, rows].T)            # [256, 1024]

    # ebs = exp(bias^T) with causal / key-padding zeros, packed into the
    # per-(h, c, g) blocks the device loads: [NH, NBLK, 128, GJT*IC].
    import ml_dtypes
    ebs = np.empty((NH, NBLK, KTILE, GW), dtype=ml_dtypes.bfloat16)
    live = np.triu(np.ones((T, T), dtype=bool))  # [j, i]: live iff j <= i
    for h in range(NH):
        bt = attn_bias[b, NH * hg + h].T          # [j, i]
        E = np.exp(bt, dtype=np.float32)
        E[~live] = 0.0
        if kp_mask is not None and kp_mask[b].any():
            E[kp_mask[b], :] = 0.0
        Eb = E.astype(ml_dtypes.bfloat16)
        Er = Eb.reshape(NJT, PJ, NIC, IC)         # [jt, p, c, i]
        for cc in range(NIC):
            ngrp = GJT * (cc + 1)                 # exp groups for this chunk
            njt = GJT * ngrp                      # live j-tiles (= 4*(cc+1))
            blk = Er[:njt, :, cc, :].reshape(ngrp, GJT, PJ, IC)
            ebs[h, cc * (cc + 1):cc * (cc + 1) + ngrp] = \
                blk.transpose(0, 2, 1, 3).reshape(ngrp, PJ, GW)
    ones = np.zeros((128, IC), np.float32)
    ones[0, :] = 1.0
    return {
        "xqT": xTs[("q", b)], "xkT": xTs[("k", b)], "xvT": xTs[("v", b)],
        "wqp": _pack_w(wq_s, DF), "wkp": _pack_w(wk_s, DF),
        "wvp": _pack_w(wv_aug, VC),
        "wot": wot, "ebsd": ebs, "bqk": np.ascontiguousarray(bqk),
        "onesd": _bf16(ones),
    }


def kernel(query, key, value, attn_bias, key_padding_mask,
           wq, bq, wk, bk, wv, bv, wo, bo):
    global LAST_EXEC_NS, LAST_RESULTS
    from concourse.bass_utils import run_bass_kernel_spmd

    query = np.asarray(query, np.float32)
    key = np.asarray(key, np.float32)
    value = np.asarray(value, np.float32)
    attn_bias = np.asarray(attn_bias, np.float32)
    kp = np.asarray(key_padding_mask).astype(bool)
    wq, bq = np.asarray(wq, np.float32), np.asarray(bq, np.float32)
    wk, bk = np.asarray(wk, np.float32), np.asarray(bk, np.float32)
    wv, bv = np.asarray(wv, np.float32), np.asarray(bv, np.float32)
    wo, bo = np.asarray(wo, np.float32), np.asarray(bo, np.float32)

    if "nc" not in _STATE:
        _STATE["nc"] = _build_nc()
    nc = _STATE["nc"]

    xTs = {}
    for tag, arr in (("q", query), ("k", key), ("v", value)):
        for b in range(B):
            xT = _bf16(arr[b].T)                  # [D, T]
            xTs[(tag, b)] = np.ascontiguousarray(
                xT.reshape(NKT, KTILE, NIC, IC).transpose(2, 1, 0, 3)
                .reshape(NIC, KTILE, NKT * IC))

    from concurrent.futures import ThreadPoolExecutor
    with ThreadPoolExecutor(NCORES) as ex:
        in_maps = list(ex.map(
            lambda c: _prep_core(c, attn_bias, kp,
                                 wq, bq, wk, bk, wv, bv, wo, xTs),
            range(NCORES)))

    trace = os.environ.get("BASS_KERNEL_TRACE", "0") == "1"
    res = run_bass_kernel_spmd(nc, in_maps, core_ids=list(range(NCORES)),
                               trace=trace)
    LAST_EXEC_NS = res.exec_time_ns
    LAST_RESULTS = res

    outp = np.empty((B, T, D), np.float32)
    for b in range(B):
        acc = res.results[4 * b]["out"].astype(np.float32)
        for g in range(1, 4):
            acc = acc + res.results[4 * b + g]["out"].astype(np.float32)
        outp[b] = acc + bo
    return outp
